# revision 47
# baseline (speedup 1.0000x reference)
"""Bidirectional LSTM Trainium2 Bass kernel — gates-transposed layout.

Problem: T=128, B=128, IN=512, H=512, OUT=512 (fp32 reference).
Sharding: data-parallel over batch + direction-parallel:
  cores 0-3: forward LSTM, batch slices 0:32, 32:64, 64:96, 96:128
  cores 4-7: backward LSTM (time-reversed x), same batch slices

Key idea vs the previous kernel: keep EVERYTHING transposed — gates,
c, h live as [feature-on-partition, batch-free] tiles. The recurrence
matmul then uses W_hh^T blocks as the STATIONARY operand and h^T
(only BL=32 columns) as the MOVING operand, so each of the 64 matmuls
per step costs N=32 rows instead of streaming the 512-wide W_hh
(N=512) — a 4x reduction in PE cycles. It also kills the per-step h
transposes entirely (h^T is what the cell update naturally produces)
and the xw seed matmuls (phase 1 computes xw^T directly INTO the same
PSUM bank the recurrence accumulates into).

Layout per step t: one PSUM bank holds gates^T [128, 16, 32] fp32,
16 gate-tiles x 32 batch. Gate-tile order (after host-side row perm):
  [i0 i1 f0 f1 o0 o1 g0 g1 | i2 i3 f2 f3 o2 o3 g2 g3]
(half h covers hidden units 256h:256h+256) so one sigmoid covers
i,i,f,f,o,o contiguously per half and one tanh covers g,g.

Per step: bias seed (4 matmuls, K=4 selection), phase-1 xw^T (64
matmuls N=32, emitted LOOKAHEAD steps early), W-MMs (64 matmuls N=32),
activations on ScalarE, c-update on VectorE, h-mul on GPSIMD writing
h^T straight into the SBUF history consumed by the next step's W-MMs
and by phase 3 (out^T = W_lin^T-blocks @ h^T, chunked, DMA'd per
chunk). Host combines: out = out_fwd + flip_t(out_bwd) + b_lin.
"""

import sys

sys.path.insert(0, "/opt/trn_rl_repo")

import functools
import os

import ml_dtypes
import numpy as np

import concourse.bass as bass
import concourse.tile as tile
from concourse import bacc, mybir
from concourse.bass_utils import run_bass_kernel_spmd

T, B, IN, H, OUT = 128, 128, 512, 512, 512
NCORES = 8
BL = B // 4  # batch per core (4 cores per direction)
G4 = 4 * H  # 2048 gate rows (transposed: gate-on-partition)
KT = IN // 128  # 4 k-tiles of 128
NGT = G4 // 128  # 16 gate tiles of 128
TCH = T // 4  # 32 column-chunks of 128 (4 steps x 32 batch)
NC_COLS = T * BL  # 4096 (t*32+b) columns

LOOKAHEAD = int(os.environ.get("LSTM_LOOKAHEAD", "1"))
RING = int(os.environ.get("LSTM_RING", "6"))  # psum gates ring (banks)
# half processed FIRST on Act/DVE each step (the other inherits queue lag)
QFIRST = int(os.environ.get("LSTM_QFIRST", "1"))
TC_EARLY = os.environ.get("LSTM_TC_EARLY", "0") == "1"
# merged: one chain per step, gate tiles [i x4, f x4, o x4, g x4], 3 Act insts
MERGED = os.environ.get("LSTM_MERGED", "1") == "1"
FC_POOL = os.environ.get("LSTM_FC_POOL", "0") == "1"
# fp8e4m3 DoubleRow recurrent matmul (W_hh and the recurrence copy of h in
# fp8; phase-3 consumes a separate bf16 h)
FP8WMM = os.environ.get("LSTM_FP8WMM", "1") == "1"

BF16 = mybir.dt.bfloat16
FP32 = mybir.dt.float32
FP8 = mybir.dt.float8e4
AF = mybir.ActivationFunctionType
DROW = mybir.MatmulPerfMode.DoubleRow


def build_nc(reps=1):
    nc = bacc.Bacc(None, target_bir_lowering=False)
    xT = nc.dram_tensor("xT", [128, KT, NC_COLS], BF16, kind="ExternalInput")
    wihT = nc.dram_tensor("wihT", [128, KT, G4], BF16, kind="ExternalInput")
    whhT = nc.dram_tensor("whhT", [128, KT, G4], FP8 if FP8WMM else BF16,
                          kind="ExternalInput")
    wlinT = nc.dram_tensor("wlinT", [128, KT, OUT], BF16, kind="ExternalInput")
    if MERGED:
        bias_if = nc.dram_tensor("bias_if", [8, 128], BF16, kind="ExternalInput")
        bias_o = nc.dram_tensor("bias_o", [4, 128], BF16, kind="ExternalInput")
        bias_g = nc.dram_tensor("bias_g", [4, 128], BF16, kind="ExternalInput")
        sel4 = nc.dram_tensor("sel4", [4, 4 * BL], BF16, kind="ExternalInput")
        sel8 = nc.dram_tensor("sel8", [8, 8 * BL], BF16, kind="ExternalInput")
    else:
        biasm = nc.dram_tensor("biasm", [16, 128], BF16, kind="ExternalInput")
        sel16 = nc.dram_tensor("sel16", [16, NGT * BL], BF16, kind="ExternalInput")
    outp = nc.dram_tensor("outp", [128, 4, NC_COLS], FP32, kind="ExternalOutput")
    debug_t0 = os.environ.get("LSTM_DEBUG_T0") == "1"
    if debug_t0:
        dbg_gates = nc.dram_tensor("dbg_gates", [128, NGT, BL], FP32, kind="ExternalOutput")
        dbg_h = nc.dram_tensor("dbg_h", [128, KT, BL], FP32, kind="ExternalOutput")

    with tile.TileContext(nc) as tc:
        with (
            tc.tile_pool(name="const", bufs=1) as constp,
            tc.tile_pool(name="xring", bufs=4) as xring,
            tc.tile_pool(name="acts", bufs=3) as actsp,
            tc.tile_pool(name="tmps", bufs=2) as tmpsp,
            tc.tile_pool(name="outsb", bufs=3) as outsbp,
            tc.tile_pool(
                name="gates", bufs=(2 if MERGED else RING), space="PSUM"
            ) as gatesp,
            tc.tile_pool(name="ps3", bufs=2, space="PSUM") as ps3,
        ):
            wih_sb = constp.tile([128, KT, G4], BF16)
            nc.sync.dma_start(wih_sb[:], wihT[:])
            if MERGED:
                bias_if_sb = constp.tile([8, 128], BF16)
                nc.sync.dma_start(bias_if_sb[:], bias_if[:])
                bias_o_sb = constp.tile([4, 128], BF16)
                nc.sync.dma_start(bias_o_sb[:], bias_o[:])
                bias_g_sb = constp.tile([4, 128], BF16)
                nc.sync.dma_start(bias_g_sb[:], bias_g[:])
                sel4_sb = constp.tile([4, 4 * BL], BF16)
                nc.sync.dma_start(sel4_sb[:], sel4[:])
                sel8_sb = constp.tile([8, 8 * BL], BF16)
                nc.sync.dma_start(sel8_sb[:], sel8[:])
            else:
                biasm_sb = constp.tile([16, 128], BF16)
                nc.sync.dma_start(biasm_sb[:], biasm[:])
                sel16_sb = constp.tile([16, NGT * BL], BF16)
                nc.sync.dma_start(sel16_sb[:], sel16[:])
            whh_sb = constp.tile([128, KT, G4], FP8 if FP8WMM else BF16)
            nc.sync.dma_start(whh_sb[:], whhT[:])
            wlin_sb = constp.tile([128, KT, OUT], BF16)
            nc.sync.dma_start(wlin_sb[:], wlinT[:])
            # h^T history: [128, k-tile, t*32+b]; written per (half, step),
            # read by next step's W-MMs and by phase 3 (subtile deps).
            hT_sb = constp.tile([128, KT, NC_COLS], BF16)
            # fp8 copy of h for the DoubleRow recurrent matmul
            hT_f8 = (
                constp.tile([128, KT, NC_COLS], FP8, name="hT_f8")
                if FP8WMM
                else None
            )
            # cell state per half, [128, 2 k-tiles, 32] fp32
            if MERGED:
                c_half = [constp.tile([128, 4, BL], FP32, name="c0")]
            else:
                c_half = [
                    constp.tile([128, 2, BL], FP32, name=f"c{q}") for q in range(2)
                ]

            for _rep in range(reps):
                for cq in c_half:
                    nc.vector.memset(cq[:], 0.0)
                banks = {}
                xch_tiles = {}

                def ensure_xchunk(ch):
                    if ch not in xch_tiles:
                        xt = xring.tile([128, KT, 128], BF16, tag="xch", name="xch")
                        nc.sync.dma_start(xt[:], xT[:, :, 128 * ch : 128 * ch + 128])
                        xch_tiles[ch] = xt
                    return xch_tiles[ch]

                def bank_slot(bank, gt):
                    # gt -> (tile, local index). MERGED: 3 tiles (if / o / g)
                    # so dependency tracking (tile-granular) separates the
                    # gate groups; non-merged: one tile.
                    if not MERGED:
                        return bank, gt
                    if gt < 8:
                        return bank[0], gt
                    if gt < 12:
                        return bank[1], gt - 8
                    return bank[2], gt - 12

                def emit_ph1(s):
                    ch, ti = s // 4, s % 4
                    xt = ensure_xchunk(ch)
                    # one start=True seed per PSUM bank (start zeroes the
                    # whole bank, so exactly one per bank)
                    if MERGED:
                        bif = gatesp.tile([128, 8, BL], FP32, tag="bif", name="bif")
                        bo = gatesp.tile([128, 4, BL], FP32, tag="bo", name="bo")
                        bg = gatesp.tile([128, 4, BL], FP32, tag="bg", name="bg")
                        bank = (bif, bo, bg)
                        for tile_, bias_sb, sel_sb in (
                            (bg, bias_g_sb, sel4_sb),
                            (bif, bias_if_sb, sel8_sb),
                            (bo, bias_o_sb, sel4_sb),
                        ):
                            nc.tensor.matmul(
                                tile_[:],
                                bias_sb[:],
                                sel_sb[:],
                                start=True,
                                stop=False,
                                skip_group_check=True,
                            )
                    else:
                        bank = gatesp.tile(
                            [128, NGT, BL], FP32, tag="bank", name="bank"
                        )
                        nc.tensor.matmul(
                            bank[:],
                            biasm_sb[:],
                            sel16_sb[:],
                            start=True,
                            stop=False,
                            skip_group_check=True,
                        )
                    banks[s] = bank
                    gt_order = (
                        list(range(12, 16)) + list(range(8)) + list(range(8, 12))
                        if MERGED
                        else range(NGT)
                    )
                    for k in range(KT):
                        for gt in gt_order:
                            tile_, li = bank_slot(bank, gt)
                            nc.tensor.matmul(
                                tile_[:, li, :],
                                wih_sb[:, k, 128 * gt : 128 * gt + 128],
                                xt[:, k, 32 * ti : 32 * ti + 32],
                                start=False,
                                stop=(s == 0 and k == KT - 1),
                                skip_group_check=True,
                            )

                def emit_wmm(t):
                    bank = banks[t]
                    cols = slice(32 * (t - 1), 32 * (t - 1) + 32)
                    if FP8WMM:
                        # fp8e4m3 DoubleRow: one matmul per (gate-tile,
                        # k-pair) contracts K=256 at 0.5 cycles/row
                        for rng in (range(0, 8), range(12, 16), range(8, 12)):
                            for j in range(KT // 2):
                                for gt in rng:
                                    tile_, li = bank_slot(bank, gt)
                                    nc.tensor.matmul(
                                        tile_[:, li, :],
                                        whh_sb[:, 2 * j : 2 * j + 2,
                                               128 * gt : 128 * gt + 128],
                                        hT_f8[:, 2 * j : 2 * j + 2, cols],
                                        start=False,
                                        stop=(j == KT // 2 - 1),
                                        perf_mode=DROW,
                                        skip_group_check=True,
                                    )
                        return
                    # k-blocks of the half produced EARLY (QFIRST) run first;
                    # within the late half's k-blocks, the QFIRST half's gate
                    # tiles close first so its activations unblock earliest.
                    if MERGED:
                        # gate-group major: i+f first (sig_if has the longest
                        # downstream chain), then g, then o (only needed by
                        # the h-mul at the chain end)
                        korder = [
                            (k, rng)
                            for rng in (range(0, 8), range(12, 16), range(8, 12))
                            for k in range(KT)
                        ]
                    else:
                        kA = (2, 3) if QFIRST == 1 else (0, 1)  # hT of QFIRST
                        kB = (0, 1) if QFIRST == 1 else (2, 3)
                        gF = range(8, NGT) if QFIRST == 1 else range(8)
                        gS = range(8) if QFIRST == 1 else range(8, NGT)
                        korder = [
                            (kA[0], range(NGT)),
                            (kA[1], range(NGT)),
                            (kB[0], gF),
                            (kB[1], gF),
                            (kB[0], gS),
                            (kB[1], gS),
                        ]
                    for k, gts in korder:
                        for gt in gts:
                            tile_, li = bank_slot(bank, gt)
                            nc.tensor.matmul(
                                tile_[:, li, :],
                                whh_sb[:, k, 128 * gt : 128 * gt + 128],
                                hT_sb[:, k, cols],
                                start=False,
                                stop=(k == KT - 1),
                                skip_group_check=True,
                            )

                def emit_cell(t):
                    bank = banks.pop(t)
                    if debug_t0 and t == 0 and not MERGED:
                        gsb = constp.tile([128, NGT, BL], FP32, name="gsb")
                        nc.vector.tensor_copy(gsb[:], bank[:])
                        nc.sync.dma_start(dbg_gates[:], gsb[:])
                    if MERGED:
                        ahm = actsp.tile([128, 8, BL], BF16, tag="ahm", name="ahm")
                        aho = actsp.tile([128, 4, BL], BF16, tag="aho", name="aho")
                        agm = actsp.tile([128, 4, BL], BF16, tag="agm", name="agm")
                        tcm = actsp.tile([128, 4, BL], BF16, tag="tcm", name="tcm")
                        fcm = tmpsp.tile([128, 4, BL], FP32, tag="fcm", name="fcm")
                        igm = tmpsp.tile([128, 4, BL], FP32, tag="igm", name="igm")
                        cq = c_half[0]
                        bif, bo, bg = bank
                        nc.scalar.activation(agm[:], bg[:], AF.Tanh)
                        nc.scalar.activation(ahm[:], bif[:], AF.Sigmoid)
                        if FC_POOL:
                            nc.gpsimd.tensor_mul(fcm[:], ahm[:, 4:8, :], cq[:])
                        else:
                            nc.vector.tensor_mul(fcm[:], ahm[:, 4:8, :], cq[:])
                        nc.vector.tensor_mul(igm[:], ahm[:, 0:4, :], agm[:])
                        nc.vector.tensor_add(cq[:], fcm[:], igm[:])
                        # o-sigmoid off the critical chain: runs during the
                        # c-add/tanh window
                        nc.scalar.activation(aho[:], bo[:], AF.Sigmoid)
                        nc.scalar.activation(tcm[:], cq[:], AF.Tanh)
                        if FP8WMM:
                            # chain-critical fp8 h for the recurrence; bf16
                            # h for phase 3 computed off-chain on GPSIMD
                            nc.vector.tensor_mul(
                                hT_f8[:, :, 32 * t : 32 * t + 32], aho[:], tcm[:]
                            )
                            nc.gpsimd.tensor_mul(
                                hT_sb[:, :, 32 * t : 32 * t + 32], aho[:], tcm[:]
                            )
                        else:
                            nc.vector.tensor_mul(
                                hT_sb[:, :, 32 * t : 32 * t + 32], aho[:], tcm[:]
                            )
                        return
                    ah, ag, tct = {}, {}, {}
                    qorder = (QFIRST, 1 - QFIRST)

                    def q_head(q):
                        ah[q] = actsp.tile(
                            [128, 6, BL], BF16, tag=f"ah{q}", name=f"ah{q}"
                        )
                        ag[q] = actsp.tile(
                            [128, 2, BL], BF16, tag=f"ag{q}", name=f"ag{q}"
                        )
                        tct[q] = actsp.tile(
                            [128, 2, BL], BF16, tag=f"tc{q}", name=f"tc{q}"
                        )
                        fc = tmpsp.tile([128, 2, BL], FP32, tag=f"fc{q}", name=f"fc{q}")
                        ig = tmpsp.tile([128, 2, BL], FP32, tag=f"ig{q}", name=f"ig{q}")
                        nc.scalar.activation(
                            ag[q][:], bank[:, 8 * q + 6 : 8 * q + 8, :], AF.Tanh
                        )
                        nc.scalar.activation(
                            ah[q][:], bank[:, 8 * q : 8 * q + 6, :], AF.Sigmoid
                        )
                        # fc on GPSIMD in parallel with ig on DVE
                        if FC_POOL:
                            nc.gpsimd.tensor_mul(fc[:], ah[q][:, 2:4, :], c_half[q][:])
                        else:
                            nc.vector.tensor_mul(fc[:], ah[q][:, 2:4, :], c_half[q][:])
                        nc.vector.tensor_mul(ig[:], ah[q][:, 0:2, :], ag[q][:])
                        nc.vector.tensor_add(c_half[q][:], fc[:], ig[:])

                    def q_tail(q):
                        nc.scalar.activation(tct[q][:], c_half[q][:], AF.Tanh)
                        nc.vector.tensor_mul(
                            hT_sb[:, 2 * q : 2 * q + 2, 32 * t : 32 * t + 32],
                            ah[q][:, 4:6, :],
                            tct[q][:],
                        )

                    if TC_EARLY:
                        q_head(qorder[0])
                        q_tail(qorder[0])
                        q_head(qorder[1])
                        q_tail(qorder[1])
                    else:
                        q_head(qorder[0])
                        q_head(qorder[1])
                        q_tail(qorder[0])
                        q_tail(qorder[1])

                def emit_ph3(ch):
                    po = ps3.tile([128, 4, 128], FP32, tag="po", name="po")
                    cols = slice(128 * ch, 128 * ch + 128)
                    for ot in range(4):
                        for k in range(KT):
                            nc.tensor.matmul(
                                po[:, ot, :],
                                wlin_sb[:, k, 128 * ot : 128 * ot + 128],
                                hT_sb[:, k, cols],
                                start=(ot == 0 and k == 0),
                                stop=(k == KT - 1),
                                skip_group_check=True,
                            )
                    ob = outsbp.tile([128, 4, 128], FP32, tag="ob", name="ob")
                    nc.vector.tensor_copy(ob[:], po[:])
                    nc.sync.dma_start(outp[:, :, cols], ob[:])

                for s in range(LOOKAHEAD):
                    emit_ph1(s)
                for t in range(T):
                    if debug_t0 and t == 1:
                        hsb = constp.tile([128, KT, BL], FP32, name="hsb")
                        nc.vector.tensor_copy(hsb[:], hT_sb[:, :, 0:BL])
                        nc.sync.dma_start(dbg_h[:], hsb[:])
                    if t > 0:
                        emit_wmm(t)
                    emit_cell(t)
                    if t + LOOKAHEAD < T:
                        emit_ph1(t + LOOKAHEAD)
                    if t % 4 == 2 and t >= 4:
                        emit_ph3(t // 4 - 1)
                emit_ph3(TCH - 1)
    nc.compile()
    return nc


@functools.lru_cache(maxsize=1)
def _program():
    return build_nc()


def _gate_perm():
    # PyTorch gate row order: i (0:H), f (H:2H), g (2H:3H), o (3H:4H).
    # Non-merged: per half h tiles [i(2h) i(2h+1) f f o o g g].
    # Merged: tiles [i0 i1 i2 i3 f0..f3 o0..o3 g0..g3].
    off = {"i": 0, "f": H, "g": 2 * H, "o": 3 * H}
    perm = []
    if MERGED:
        for gate in ("i", "f", "o", "g"):
            perm += list(range(off[gate], off[gate] + H))
    else:
        for h in range(2):
            for gate in ("i", "f", "o", "g"):
                for j in (2 * h, 2 * h + 1):
                    perm += list(
                        range(off[gate] + 128 * j, off[gate] + 128 * j + 128)
                    )
    return np.asarray(perm)


def _prep_core(x, W_ih, W_hh, b_ih, b_hh, W_lin, direction, bs):
    perm = _gate_perm()
    bf16 = ml_dtypes.bfloat16
    xs = np.asarray(x)[:, bs : bs + BL, :]
    if direction == 1:
        xs = xs[::-1]
    # xT[p, k, t*32+b] = xs[t, b, 128k+p]
    xTl = np.ascontiguousarray(
        xs.reshape(T, BL, KT, 128).transpose(3, 2, 0, 1).reshape(128, KT, NC_COLS)
    ).astype(bf16)
    Wp_ih = np.asarray(W_ih)[perm]  # [G4, IN]
    Wp_hh = np.asarray(W_hh)[perm]  # [G4, H]
    wihT = np.ascontiguousarray(
        Wp_ih.T.reshape(KT, 128, G4).transpose(1, 0, 2)
    ).astype(bf16)
    whhT = np.ascontiguousarray(
        Wp_hh.T.reshape(KT, 128, G4).transpose(1, 0, 2)
    ).astype(ml_dtypes.float8_e4m3 if FP8WMM else bf16)
    bp = (np.asarray(b_ih) + np.asarray(b_hh))[perm].astype(np.float32)
    # bias seed matmuls: out[p, gt_local, b] = bias[128*gt + p]
    Wl = np.asarray(W_lin)[:, direction * H : (direction + 1) * H]  # [OUT, H]
    wlinT = np.ascontiguousarray(
        Wl.T.reshape(KT, 128, OUT).transpose(1, 0, 2)
    ).astype(bf16)
    out = {"xT": xTl, "wihT": wihT, "whhT": whhT, "wlinT": wlinT}
    if MERGED:
        out["bias_if"] = np.ascontiguousarray(bp[0:1024].reshape(8, 128)).astype(bf16)
        out["bias_o"] = np.ascontiguousarray(bp[1024:1536].reshape(4, 128)).astype(
            bf16
        )
        out["bias_g"] = np.ascontiguousarray(bp[1536:2048].reshape(4, 128)).astype(
            bf16
        )
        out["sel4"] = np.ascontiguousarray(np.repeat(np.eye(4, dtype=bf16), BL, 1))
        out["sel8"] = np.ascontiguousarray(np.repeat(np.eye(8, dtype=bf16), BL, 1))
    else:
        out["biasm"] = np.ascontiguousarray(bp.reshape(16, 128)).astype(bf16)
        out["sel16"] = np.ascontiguousarray(np.repeat(np.eye(16, dtype=bf16), BL, 1))
    return out


def run_cores(inputs, trace=False):
    """Build per-core in_maps, run on 8 cores, return BassKernelResults."""
    in_maps = []
    for core in range(NCORES):
        direction = core // 4
        bs = (core % 4) * BL
        wk = "f" if direction == 0 else "b"
        in_maps.append(
            _prep_core(
                inputs["x"],
                inputs[f"W_ih_{wk}"],
                inputs[f"W_hh_{wk}"],
                inputs[f"b_ih_{wk}"],
                inputs[f"b_hh_{wk}"],
                inputs["W_lin"],
                direction,
                bs,
            )
        )
    nc = _program()
    return run_bass_kernel_spmd(nc, in_maps, list(range(NCORES)), trace=trace)


def _assemble(results, b_lin):
    # per-core outp: [128, 4, T*BL]; part[t, b, 128*ot+p] = outp[p, ot, 32t+b]
    out = np.zeros((T, B, OUT), np.float32)
    for core in range(NCORES):
        direction = core // 4
        bs = (core % 4) * BL
        dev = np.asarray(results[core]["outp"], np.float32)  # [128, 4, 4096]
        part = dev.reshape(128, 4, T, BL).transpose(2, 3, 1, 0).reshape(T, BL, OUT)
        if direction == 1:
            part = part[::-1]
        out[:, bs : bs + BL, :] += part
    out += np.asarray(b_lin, np.float32)[None, None, :]
    return out


def kernel(**inputs):
    res = run_cores(inputs, trace=False)
    return _assemble(res.results, inputs["b_lin"])


# revision 50
# speedup vs baseline: 1.0468x; 1.0468x over previous
"""Bidirectional LSTM Trainium2 Bass kernel — gates-transposed layout.

Problem: T=128, B=128, IN=512, H=512, OUT=512 (fp32 reference).
Sharding: data-parallel over batch + direction-parallel:
  cores 0-3: forward LSTM, batch slices 0:32, 32:64, 64:96, 96:128
  cores 4-7: backward LSTM (time-reversed x), same batch slices

Key idea vs the previous kernel: keep EVERYTHING transposed — gates,
c, h live as [feature-on-partition, batch-free] tiles. The recurrence
matmul then uses W_hh^T blocks as the STATIONARY operand and h^T
(only BL=32 columns) as the MOVING operand, so each of the 64 matmuls
per step costs N=32 rows instead of streaming the 512-wide W_hh
(N=512) — a 4x reduction in PE cycles. It also kills the per-step h
transposes entirely (h^T is what the cell update naturally produces)
and the xw seed matmuls (phase 1 computes xw^T directly INTO the same
PSUM bank the recurrence accumulates into).

Layout per step t: one PSUM bank holds gates^T [128, 16, 32] fp32,
16 gate-tiles x 32 batch. Gate-tile order (after host-side row perm):
  [i0 i1 f0 f1 o0 o1 g0 g1 | i2 i3 f2 f3 o2 o3 g2 g3]
(half h covers hidden units 256h:256h+256) so one sigmoid covers
i,i,f,f,o,o contiguously per half and one tanh covers g,g.

Per step: bias seed (4 matmuls, K=4 selection), phase-1 xw^T (64
matmuls N=32, emitted LOOKAHEAD steps early), W-MMs (64 matmuls N=32),
activations on ScalarE, c-update on VectorE, h-mul on GPSIMD writing
h^T straight into the SBUF history consumed by the next step's W-MMs
and by phase 3 (out^T = W_lin^T-blocks @ h^T, chunked, DMA'd per
chunk). Host combines: out = out_fwd + flip_t(out_bwd) + b_lin.
"""

import sys

sys.path.insert(0, "/opt/trn_rl_repo")

import functools
import os

import ml_dtypes
import numpy as np

import concourse.bass as bass
import concourse.tile as tile
from concourse import bacc, mybir
from concourse.bass_utils import run_bass_kernel_spmd

T, B, IN, H, OUT = 128, 128, 512, 512, 512
NCORES = 8
BL = B // 4  # batch per core (4 cores per direction)
G4 = 4 * H  # 2048 gate rows (transposed: gate-on-partition)
KT = IN // 128  # 4 k-tiles of 128
NGT = G4 // 128  # 16 gate tiles of 128
TCH = T // 4  # 32 column-chunks of 128 (4 steps x 32 batch)
NC_COLS = T * BL  # 4096 (t*32+b) columns

LOOKAHEAD = int(os.environ.get("LSTM_LOOKAHEAD", "1"))
RING = int(os.environ.get("LSTM_RING", "6"))  # psum gates ring (banks)
# half processed FIRST on Act/DVE each step (the other inherits queue lag)
QFIRST = int(os.environ.get("LSTM_QFIRST", "1"))
TC_EARLY = os.environ.get("LSTM_TC_EARLY", "0") == "1"
# merged: one chain per step, gate tiles [i x4, f x4, o x4, g x4], 3 Act insts
MERGED = os.environ.get("LSTM_MERGED", "1") == "1"
FC_POOL = os.environ.get("LSTM_FC_POOL", "0") == "1"
# fp8e4m3 DoubleRow recurrent matmul (W_hh and the recurrence copy of h in
# fp8; phase-3 consumes a separate bf16 h)
FP8WMM = os.environ.get("LSTM_FP8WMM", "1") == "1"

BF16 = mybir.dt.bfloat16
FP16 = mybir.dt.float16
FP32 = mybir.dt.float32
FP8 = mybir.dt.float8e4
AF = mybir.ActivationFunctionType
DROW = mybir.MatmulPerfMode.DoubleRow


def build_nc(reps=1):
    nc = bacc.Bacc(None, target_bir_lowering=False)
    xT = nc.dram_tensor("xT", [128, KT, NC_COLS], BF16, kind="ExternalInput")
    wihT = nc.dram_tensor("wihT", [128, KT, G4], BF16, kind="ExternalInput")
    whhT = nc.dram_tensor("whhT", [128, KT, G4], FP8 if FP8WMM else BF16,
                          kind="ExternalInput")
    wlinT = nc.dram_tensor("wlinT", [128, KT, OUT], BF16, kind="ExternalInput")
    if MERGED:
        bias_if = nc.dram_tensor("bias_if", [8, 128], BF16, kind="ExternalInput")
        bias_o = nc.dram_tensor("bias_o", [4, 128], BF16, kind="ExternalInput")
        bias_g = nc.dram_tensor("bias_g", [4, 128], BF16, kind="ExternalInput")
        sel4 = nc.dram_tensor("sel4", [4, 4 * BL], BF16, kind="ExternalInput")
        sel8 = nc.dram_tensor("sel8", [8, 8 * BL], BF16, kind="ExternalInput")
    else:
        biasm = nc.dram_tensor("biasm", [16, 128], BF16, kind="ExternalInput")
        sel16 = nc.dram_tensor("sel16", [16, NGT * BL], BF16, kind="ExternalInput")
    outp = nc.dram_tensor("outp", [128, 4, NC_COLS], FP32, kind="ExternalOutput")
    debug_t0 = os.environ.get("LSTM_DEBUG_T0") == "1"
    if debug_t0:
        dbg_gates = nc.dram_tensor("dbg_gates", [128, NGT, BL], FP32, kind="ExternalOutput")
        dbg_h = nc.dram_tensor("dbg_h", [128, KT, BL], FP32, kind="ExternalOutput")

    with tile.TileContext(nc) as tc:
        with (
            tc.tile_pool(name="const", bufs=1) as constp,
            tc.tile_pool(name="xring", bufs=4) as xring,
            tc.tile_pool(name="acts", bufs=3) as actsp,
            tc.tile_pool(name="tmps", bufs=2) as tmpsp,
            tc.tile_pool(name="outsb", bufs=3) as outsbp,
            tc.tile_pool(
                name="gates", bufs=(2 if MERGED else RING), space="PSUM"
            ) as gatesp,
            tc.tile_pool(name="ps3", bufs=2, space="PSUM") as ps3,
        ):
            wih_sb = constp.tile([128, KT, G4], BF16)
            nc.sync.dma_start(wih_sb[:], wihT[:])
            if MERGED:
                bias_if_sb = constp.tile([8, 128], BF16)
                nc.sync.dma_start(bias_if_sb[:], bias_if[:])
                bias_o_sb = constp.tile([4, 128], BF16)
                nc.sync.dma_start(bias_o_sb[:], bias_o[:])
                bias_g_sb = constp.tile([4, 128], BF16)
                nc.sync.dma_start(bias_g_sb[:], bias_g[:])
                sel4_sb = constp.tile([4, 4 * BL], BF16)
                nc.sync.dma_start(sel4_sb[:], sel4[:])
                sel8_sb = constp.tile([8, 8 * BL], BF16)
                nc.sync.dma_start(sel8_sb[:], sel8[:])
            else:
                biasm_sb = constp.tile([16, 128], BF16)
                nc.sync.dma_start(biasm_sb[:], biasm[:])
                sel16_sb = constp.tile([16, NGT * BL], BF16)
                nc.sync.dma_start(sel16_sb[:], sel16[:])
            whh_sb = constp.tile([128, KT, G4], FP8 if FP8WMM else BF16)
            nc.sync.dma_start(whh_sb[:], whhT[:])
            wlin_sb = constp.tile([128, KT, OUT], BF16)
            nc.sync.dma_start(wlin_sb[:], wlinT[:])
            # h^T history: [128, k-tile, t*32+b]; written per (half, step),
            # read by next step's W-MMs and by phase 3 (subtile deps).
            hT_sb = constp.tile([128, KT, NC_COLS], BF16)
            # fp8 copy of h for the DoubleRow recurrent matmul
            hT_f8 = (
                constp.tile([128, KT, NC_COLS], FP8, name="hT_f8")
                if FP8WMM
                else None
            )
            # cell state per half, [128, 2 k-tiles, 32] fp32
            if MERGED:
                # fp16 cell state: 2-byte dtype enables the DVE 2x_1p mode
                # on the fc/ig/c TensorTensor chain
                c_half = [constp.tile([128, 4, BL], FP16, name="c0")]
            else:
                c_half = [
                    constp.tile([128, 2, BL], FP32, name=f"c{q}") for q in range(2)
                ]

            for _rep in range(reps):
                for cq in c_half:
                    nc.vector.memset(cq[:], 0.0)
                banks = {}
                xch_tiles = {}

                def ensure_xchunk(ch):
                    if ch not in xch_tiles:
                        xt = xring.tile([128, KT, 128], BF16, tag="xch", name="xch")
                        nc.sync.dma_start(xt[:], xT[:, :, 128 * ch : 128 * ch + 128])
                        xch_tiles[ch] = xt
                    return xch_tiles[ch]

                def bank_slot(bank, gt):
                    # gt -> (tile, local index). MERGED: 3 tiles (if / o / g)
                    # so dependency tracking (tile-granular) separates the
                    # gate groups; non-merged: one tile.
                    if not MERGED:
                        return bank, gt
                    if gt < 8:
                        return bank[0], gt
                    if gt < 12:
                        return bank[1], gt - 8
                    return bank[2], gt - 12

                def emit_ph1(s):
                    ch, ti = s // 4, s % 4
                    xt = ensure_xchunk(ch)
                    # one start=True seed per PSUM bank (start zeroes the
                    # whole bank, so exactly one per bank)
                    if MERGED:
                        bif = gatesp.tile([128, 8, BL], FP32, tag="bif", name="bif")
                        bo = gatesp.tile([128, 4, BL], FP32, tag="bo", name="bo")
                        bg = gatesp.tile([128, 4, BL], FP32, tag="bg", name="bg")
                        bank = (bif, bo, bg)
                        for tile_, bias_sb, sel_sb in (
                            (bg, bias_g_sb, sel4_sb),
                            (bif, bias_if_sb, sel8_sb),
                            (bo, bias_o_sb, sel4_sb),
                        ):
                            nc.tensor.matmul(
                                tile_[:],
                                bias_sb[:],
                                sel_sb[:],
                                start=True,
                                stop=False,
                                skip_group_check=True,
                            )
                    else:
                        bank = gatesp.tile(
                            [128, NGT, BL], FP32, tag="bank", name="bank"
                        )
                        nc.tensor.matmul(
                            bank[:],
                            biasm_sb[:],
                            sel16_sb[:],
                            start=True,
                            stop=False,
                            skip_group_check=True,
                        )
                    banks[s] = bank
                    gt_order = (
                        list(range(12, 16)) + list(range(8)) + list(range(8, 12))
                        if MERGED
                        else range(NGT)
                    )
                    for k in range(KT):
                        for gt in gt_order:
                            tile_, li = bank_slot(bank, gt)
                            nc.tensor.matmul(
                                tile_[:, li, :],
                                wih_sb[:, k, 128 * gt : 128 * gt + 128],
                                xt[:, k, 32 * ti : 32 * ti + 32],
                                start=False,
                                stop=(s == 0 and k == KT - 1),
                                skip_group_check=True,
                            )

                def emit_wmm(t):
                    bank = banks[t]
                    cols = slice(32 * (t - 1), 32 * (t - 1) + 32)
                    if FP8WMM:
                        # fp8e4m3 DoubleRow: one matmul per (gate-tile,
                        # k-pair) contracts K=256 at 0.5 cycles/row
                        for rng in (range(0, 8), range(12, 16), range(8, 12)):
                            for j in range(KT // 2):
                                for gt in rng:
                                    tile_, li = bank_slot(bank, gt)
                                    nc.tensor.matmul(
                                        tile_[:, li, :],
                                        whh_sb[:, 2 * j : 2 * j + 2,
                                               128 * gt : 128 * gt + 128],
                                        hT_f8[:, 2 * j : 2 * j + 2, cols],
                                        start=False,
                                        stop=(j == KT // 2 - 1),
                                        perf_mode=DROW,
                                        skip_group_check=True,
                                    )
                        return
                    # k-blocks of the half produced EARLY (QFIRST) run first;
                    # within the late half's k-blocks, the QFIRST half's gate
                    # tiles close first so its activations unblock earliest.
                    if MERGED:
                        # gate-group major: i+f first (sig_if has the longest
                        # downstream chain), then g, then o (only needed by
                        # the h-mul at the chain end)
                        korder = [
                            (k, rng)
                            for rng in (range(0, 8), range(12, 16), range(8, 12))
                            for k in range(KT)
                        ]
                    else:
                        kA = (2, 3) if QFIRST == 1 else (0, 1)  # hT of QFIRST
                        kB = (0, 1) if QFIRST == 1 else (2, 3)
                        gF = range(8, NGT) if QFIRST == 1 else range(8)
                        gS = range(8) if QFIRST == 1 else range(8, NGT)
                        korder = [
                            (kA[0], range(NGT)),
                            (kA[1], range(NGT)),
                            (kB[0], gF),
                            (kB[1], gF),
                            (kB[0], gS),
                            (kB[1], gS),
                        ]
                    for k, gts in korder:
                        for gt in gts:
                            tile_, li = bank_slot(bank, gt)
                            nc.tensor.matmul(
                                tile_[:, li, :],
                                whh_sb[:, k, 128 * gt : 128 * gt + 128],
                                hT_sb[:, k, cols],
                                start=False,
                                stop=(k == KT - 1),
                                skip_group_check=True,
                            )

                def emit_cell(t):
                    bank = banks.pop(t)
                    if debug_t0 and t == 0 and not MERGED:
                        gsb = constp.tile([128, NGT, BL], FP32, name="gsb")
                        nc.vector.tensor_copy(gsb[:], bank[:])
                        nc.sync.dma_start(dbg_gates[:], gsb[:])
                    if MERGED:
                        ahm = actsp.tile([128, 8, BL], BF16, tag="ahm", name="ahm")
                        aho = actsp.tile([128, 4, BL], BF16, tag="aho", name="aho")
                        agm = actsp.tile([128, 4, BL], BF16, tag="agm", name="agm")
                        tcm = actsp.tile([128, 4, BL], BF16, tag="tcm", name="tcm")
                        fcm = tmpsp.tile([128, 4, BL], FP16, tag="fcm", name="fcm")
                        igm = tmpsp.tile([128, 4, BL], FP16, tag="igm", name="igm")
                        cq = c_half[0]
                        bif, bo, bg = bank
                        nc.scalar.activation(agm[:], bg[:], AF.Tanh)
                        nc.scalar.activation(ahm[:], bif[:], AF.Sigmoid)
                        if FC_POOL:
                            nc.gpsimd.tensor_mul(fcm[:], ahm[:, 4:8, :], cq[:])
                        else:
                            nc.vector.tensor_mul(fcm[:], ahm[:, 4:8, :], cq[:])
                        nc.vector.tensor_mul(igm[:], ahm[:, 0:4, :], agm[:])
                        nc.vector.tensor_add(cq[:], fcm[:], igm[:])
                        # o-sigmoid off the critical chain: runs during the
                        # c-add/tanh window
                        nc.scalar.activation(aho[:], bo[:], AF.Sigmoid)
                        nc.scalar.activation(tcm[:], cq[:], AF.Tanh)
                        if FP8WMM:
                            # chain-critical fp8 h for the recurrence; bf16
                            # h for phase 3 computed off-chain on GPSIMD
                            nc.vector.tensor_mul(
                                hT_f8[:, :, 32 * t : 32 * t + 32], aho[:], tcm[:]
                            )
                            nc.gpsimd.tensor_mul(
                                hT_sb[:, :, 32 * t : 32 * t + 32], aho[:], tcm[:]
                            )
                        else:
                            nc.vector.tensor_mul(
                                hT_sb[:, :, 32 * t : 32 * t + 32], aho[:], tcm[:]
                            )
                        return
                    ah, ag, tct = {}, {}, {}
                    qorder = (QFIRST, 1 - QFIRST)

                    def q_head(q):
                        ah[q] = actsp.tile(
                            [128, 6, BL], BF16, tag=f"ah{q}", name=f"ah{q}"
                        )
                        ag[q] = actsp.tile(
                            [128, 2, BL], BF16, tag=f"ag{q}", name=f"ag{q}"
                        )
                        tct[q] = actsp.tile(
                            [128, 2, BL], BF16, tag=f"tc{q}", name=f"tc{q}"
                        )
                        fc = tmpsp.tile([128, 2, BL], FP32, tag=f"fc{q}", name=f"fc{q}")
                        ig = tmpsp.tile([128, 2, BL], FP32, tag=f"ig{q}", name=f"ig{q}")
                        nc.scalar.activation(
                            ag[q][:], bank[:, 8 * q + 6 : 8 * q + 8, :], AF.Tanh
                        )
                        nc.scalar.activation(
                            ah[q][:], bank[:, 8 * q : 8 * q + 6, :], AF.Sigmoid
                        )
                        # fc on GPSIMD in parallel with ig on DVE
                        if FC_POOL:
                            nc.gpsimd.tensor_mul(fc[:], ah[q][:, 2:4, :], c_half[q][:])
                        else:
                            nc.vector.tensor_mul(fc[:], ah[q][:, 2:4, :], c_half[q][:])
                        nc.vector.tensor_mul(ig[:], ah[q][:, 0:2, :], ag[q][:])
                        nc.vector.tensor_add(c_half[q][:], fc[:], ig[:])

                    def q_tail(q):
                        nc.scalar.activation(tct[q][:], c_half[q][:], AF.Tanh)
                        nc.vector.tensor_mul(
                            hT_sb[:, 2 * q : 2 * q + 2, 32 * t : 32 * t + 32],
                            ah[q][:, 4:6, :],
                            tct[q][:],
                        )

                    if TC_EARLY:
                        q_head(qorder[0])
                        q_tail(qorder[0])
                        q_head(qorder[1])
                        q_tail(qorder[1])
                    else:
                        q_head(qorder[0])
                        q_head(qorder[1])
                        q_tail(qorder[0])
                        q_tail(qorder[1])

                def emit_ph3(ch):
                    po = ps3.tile([128, 4, 128], FP32, tag="po", name="po")
                    cols = slice(128 * ch, 128 * ch + 128)
                    for ot in range(4):
                        for k in range(KT):
                            nc.tensor.matmul(
                                po[:, ot, :],
                                wlin_sb[:, k, 128 * ot : 128 * ot + 128],
                                hT_sb[:, k, cols],
                                start=(ot == 0 and k == 0),
                                stop=(k == KT - 1),
                                skip_group_check=True,
                            )
                    ob = outsbp.tile([128, 4, 128], FP32, tag="ob", name="ob")
                    nc.vector.tensor_copy(ob[:], po[:])
                    nc.sync.dma_start(outp[:, :, cols], ob[:])

                for s in range(LOOKAHEAD):
                    emit_ph1(s)
                for t in range(T):
                    if debug_t0 and t == 1:
                        hsb = constp.tile([128, KT, BL], FP32, name="hsb")
                        nc.vector.tensor_copy(hsb[:], hT_sb[:, :, 0:BL])
                        nc.sync.dma_start(dbg_h[:], hsb[:])
                    if t > 0:
                        emit_wmm(t)
                    emit_cell(t)
                    if t + LOOKAHEAD < T:
                        emit_ph1(t + LOOKAHEAD)
                    if t % 4 == 2 and t >= 4:
                        emit_ph3(t // 4 - 1)
                emit_ph3(TCH - 1)
    nc.compile()
    return nc


@functools.lru_cache(maxsize=1)
def _program():
    return build_nc()


def _gate_perm():
    # PyTorch gate row order: i (0:H), f (H:2H), g (2H:3H), o (3H:4H).
    # Non-merged: per half h tiles [i(2h) i(2h+1) f f o o g g].
    # Merged: tiles [i0 i1 i2 i3 f0..f3 o0..o3 g0..g3].
    off = {"i": 0, "f": H, "g": 2 * H, "o": 3 * H}
    perm = []
    if MERGED:
        for gate in ("i", "f", "o", "g"):
            perm += list(range(off[gate], off[gate] + H))
    else:
        for h in range(2):
            for gate in ("i", "f", "o", "g"):
                for j in (2 * h, 2 * h + 1):
                    perm += list(
                        range(off[gate] + 128 * j, off[gate] + 128 * j + 128)
                    )
    return np.asarray(perm)


def _prep_core(x, W_ih, W_hh, b_ih, b_hh, W_lin, direction, bs):
    perm = _gate_perm()
    bf16 = ml_dtypes.bfloat16
    xs = np.asarray(x)[:, bs : bs + BL, :]
    if direction == 1:
        xs = xs[::-1]
    # xT[p, k, t*32+b] = xs[t, b, 128k+p]
    xTl = np.ascontiguousarray(
        xs.reshape(T, BL, KT, 128).transpose(3, 2, 0, 1).reshape(128, KT, NC_COLS)
    ).astype(bf16)
    Wp_ih = np.asarray(W_ih)[perm]  # [G4, IN]
    Wp_hh = np.asarray(W_hh)[perm]  # [G4, H]
    wihT = np.ascontiguousarray(
        Wp_ih.T.reshape(KT, 128, G4).transpose(1, 0, 2)
    ).astype(bf16)
    whhT = np.ascontiguousarray(
        Wp_hh.T.reshape(KT, 128, G4).transpose(1, 0, 2)
    ).astype(ml_dtypes.float8_e4m3 if FP8WMM else bf16)
    bp = (np.asarray(b_ih) + np.asarray(b_hh))[perm].astype(np.float32)
    # bias seed matmuls: out[p, gt_local, b] = bias[128*gt + p]
    Wl = np.asarray(W_lin)[:, direction * H : (direction + 1) * H]  # [OUT, H]
    wlinT = np.ascontiguousarray(
        Wl.T.reshape(KT, 128, OUT).transpose(1, 0, 2)
    ).astype(bf16)
    out = {"xT": xTl, "wihT": wihT, "whhT": whhT, "wlinT": wlinT}
    if MERGED:
        out["bias_if"] = np.ascontiguousarray(bp[0:1024].reshape(8, 128)).astype(bf16)
        out["bias_o"] = np.ascontiguousarray(bp[1024:1536].reshape(4, 128)).astype(
            bf16
        )
        out["bias_g"] = np.ascontiguousarray(bp[1536:2048].reshape(4, 128)).astype(
            bf16
        )
        out["sel4"] = np.ascontiguousarray(np.repeat(np.eye(4, dtype=bf16), BL, 1))
        out["sel8"] = np.ascontiguousarray(np.repeat(np.eye(8, dtype=bf16), BL, 1))
    else:
        out["biasm"] = np.ascontiguousarray(bp.reshape(16, 128)).astype(bf16)
        out["sel16"] = np.ascontiguousarray(np.repeat(np.eye(16, dtype=bf16), BL, 1))
    return out


def run_cores(inputs, trace=False):
    """Build per-core in_maps, run on 8 cores, return BassKernelResults."""
    in_maps = []
    for core in range(NCORES):
        direction = core // 4
        bs = (core % 4) * BL
        wk = "f" if direction == 0 else "b"
        in_maps.append(
            _prep_core(
                inputs["x"],
                inputs[f"W_ih_{wk}"],
                inputs[f"W_hh_{wk}"],
                inputs[f"b_ih_{wk}"],
                inputs[f"b_hh_{wk}"],
                inputs["W_lin"],
                direction,
                bs,
            )
        )
    nc = _program()
    return run_bass_kernel_spmd(nc, in_maps, list(range(NCORES)), trace=trace)


def _assemble(results, b_lin):
    # per-core outp: [128, 4, T*BL]; part[t, b, 128*ot+p] = outp[p, ot, 32t+b]
    out = np.zeros((T, B, OUT), np.float32)
    for core in range(NCORES):
        direction = core // 4
        bs = (core % 4) * BL
        dev = np.asarray(results[core]["outp"], np.float32)  # [128, 4, 4096]
        part = dev.reshape(128, 4, T, BL).transpose(2, 3, 1, 0).reshape(T, BL, OUT)
        if direction == 1:
            part = part[::-1]
        out[:, bs : bs + BL, :] += part
    out += np.asarray(b_lin, np.float32)[None, None, :]
    return out


def kernel(**inputs):
    res = run_cores(inputs, trace=False)
    return _assemble(res.results, inputs["b_lin"])
